# revision 1
# baseline (speedup 1.0000x reference)
"""Fused ViT-style transformer block on 8 TRN2 NeuronCores (pipelined v2).

Sharding: data-parallel over batch (32 batches -> 4 per core). Each core runs
the full block (LN1 -> QKV -> attention -> proj -> residual -> LN2 -> MLP ->
residual) on its 4 batches. No collectives.

v2 structure: the 4 per-core batches are software-pipelined so the PE never
waits on the LayerNorm/softmax chains:
  - fc2 of batch b-1 is interleaved into batch b's attention loop (PE work
    under the ACT-bound exp stream).
  - LN1 of batch b+1 runs during batch b's o-transpose/proj window.
  - v-matmuls of batch b+1 fill the LN2(b) stats/normalize/transpose chain.
Other changes vs v1:
  - LN rstd = exp(-0.5*ln(var+eps)) so the single natural_log_exp ACT table
    serves LN + softmax + relu with zero table switches.
  - proj/fc2 biases enter PSUM as K=1 ones-row matmuls (Pool engine freed).
  - All transposes ride the DMA xbar (tail chunk padded to 128 rows; the
    garbage columns are never read), so the PE matmul stream is never
    interrupted and no PSUM bank is spent on transposes.
  - PV accumulates both heads of a pair into one PSUM bank with the softmax
    row-sum column; normalize is one reciprocal + one broadcast multiply.
"""

import time

import numpy as np
import ml_dtypes
from contextlib import ExitStack

import concourse.bass as bass
import concourse.bacc as bacc
import concourse.tile as tile
from concourse import mybir

# Constrain the ACT table-set chooser to natural_log_exp_and_others, which
# covers every activation this kernel uses (ln, exp, relu, copy, identity).
# The default chooser picks the first set containing each function, which
# thrashes table loads (~2.7us each) between the LN rstd chain and softmax.
# Set indices are preserved (other sets are emptied, not removed).
_NLE = "natural_log_exp_and_others"
_gat_orig = bacc.get_activation_tables


def _gat_nle_only(arch):
    t = _gat_orig(arch)
    assert _NLE in t
    return {k: (v if k == _NLE else set()) for k, v in t.items()}


bacc.get_activation_tables = _gat_nle_only

F32 = mybir.dt.float32
BF16 = mybir.dt.bfloat16
FP8 = mybir.dt.float8e4
AF = mybir.ActivationFunctionType
OP = mybir.AluOpType

B, N, D, H = 32, 577, 768, 12
DH = D // H            # 64
HID = 4 * D            # 3072
NCORES = 8
BPC = B // NCORES      # batches per core
P = 128
KD = D // P            # 6
KH = HID // P          # 24
NT = 5                 # token chunks per batch: 4x128 + 65
TSZ = [128, 128, 128, 128, 65]
TOF = [0, 128, 256, 384, 512]
EPS = 1e-5
SCALE = DH ** -0.5


def _bcast(ap_1d, p=P):
    """AP that reads a 1-D dram tensor broadcast across p partitions."""
    return bass.AP(
        tensor=ap_1d.tensor, offset=ap_1d.offset, ap=[[0, p]] + list(ap_1d.ap)
    )


def _bcast_free(ap, n):
    """AP with an appended stride-0 free dim of length n (broadcast)."""
    return bass.AP(tensor=ap.tensor, offset=ap.offset, ap=list(ap.ap) + [[0, n]])


def _body(ctx, tc, d):
    nc = tc.nc

    const = ctx.enter_context(tc.tile_pool(name="const", bufs=1))
    xp = ctx.enter_context(tc.tile_pool(name="xp", bufs=3))
    hbf = ctx.enter_context(tc.tile_pool(name="hbf", bufs=2))
    t6 = ctx.enter_context(tc.tile_pool(name="t6", bufs=3))
    vvp = ctx.enter_context(tc.tile_pool(name="vv", bufs=1))
    qkp = ctx.enter_context(tc.tile_pool(name="qk", bufs=4))
    esp = ctx.enter_context(tc.tile_pool(name="es", bufs=2))
    h1p = ctx.enter_context(tc.tile_pool(name="h1", bufs=1))
    w1p = ctx.enter_context(tc.tile_pool(name="w1", bufs=4))
    stat = ctx.enter_context(tc.tile_pool(name="stat", bufs=6))
    ps_mm = ctx.enter_context(tc.tile_pool(name="ps_mm", bufs=3, space="PSUM"))
    ps_pv = ctx.enter_context(tc.tile_pool(name="ps_pv", bufs=2, space="PSUM"))

    # ---- one-time constants ----
    eps_sb = const.tile([P, 1], F32)
    nc.vector.memset(eps_sb, EPS)
    nshift_sb = const.tile([P, 1], F32)
    nc.vector.memset(nshift_sb, -2.5)
    ones1 = const.tile([1, P], BF16)
    nc.vector.memset(ones1, 1.0)

    wqkv_sb = const.tile([P, KD, 3 * D], BF16)
    # v-columns first: the v matmuls are the first consumers of wqkv
    nc.gpsimd.dma_start(
        out=wqkv_sb[:, :, 2 * D :],
        in_=d["w_qkv"][:, 2 * D :].rearrange("(c p) f -> p c f", p=P),
    )
    nc.gpsimd.dma_start(
        out=wqkv_sb[:, :, : 2 * D],
        in_=d["w_qkv"][:, : 2 * D].rearrange("(c p) f -> p c f", p=P),
    )
    # wproj/wfc2 tiles are DMA'd after batch 0's v-phase so the Pool queue
    # isn't hogged at startup
    wproj_sb = const.tile([P, KD, D], BF16)
    nc.gpsimd.dma_start(
        out=wproj_sb, in_=d["w_proj"][:].rearrange("(c p) f -> p c f", p=P)
    )
    wfc2_sb = const.tile([P, KH, D], BF16)
    nc.gpsimd.dma_start(
        out=wfc2_sb, in_=d["w_fc2"][:].rearrange("(c p) f -> p c f", p=P)
    )
    bqk_sb = const.tile([P, 12], F32)
    nc.sync.dma_start(out=bqk_sb, in_=d["b_qk"][:])
    bfc1_sb = const.tile([P, KH], F32)
    nc.sync.dma_start(out=bfc1_sb, in_=d["b_fc1"][:])
    bv_sb = const.tile([P, D], BF16)
    nc.sync.dma_start(out=bv_sb, in_=_bcast(d["b_v"][:]))
    bprow = const.tile([1, D], BF16)
    nc.sync.dma_start(out=bprow, in_=_bcast(d["b_proj"][:], p=1))
    bf2row = const.tile([1, D], BF16)
    nc.sync.dma_start(out=bf2row, in_=_bcast(d["b_fc2"][:], p=1))

    x_d, y_d = d["x"], d["y"]

    NSPL = ((0, 512), (512, N))    # token-free splits (qk/scores/fc1 rhs)
    DSPL = ((0, 512), (512, D))    # feature-free splits (v/proj/fc2 out)

    def load_x(b):
        x_sb = xp.tile([P, NT, D], BF16, tag="x")
        for t in range(NT):
            nc.sync.dma_start(
                out=x_sb[: TSZ[t], t, :],
                in_=x_d[b, TOF[t] : TOF[t] + TSZ[t], :],
            )
        return x_sb

    def ln_chunk(x_sb, h_sb, t, eng=None):
        """h[:,t,:] = (x - mu) * rsqrt(var + eps); rstd via exp(-0.5*ln())."""
        ts_ = TSZ[t]
        st = stat.tile([P, 2, 6], F32, tag="bnst")
        mv = stat.tile([P, 2], F32, tag="bnmv")
        xin = x_sb[:ts_, t, :].rearrange("p (s q) -> p s q", s=2)
        for s in range(2):
            nc.vector.bn_stats(out=st[:ts_, s, :], in_=xin[:, s, :])
        nc.vector.bn_aggr(out=mv[:ts_], in_=st[:ts_])
        rstd = stat.tile([P, 1], F32, tag="rstd")
        nc.scalar.activation(
            out=rstd[:ts_], in_=mv[:ts_, 1:2], func=AF.Ln, bias=eps_sb[:ts_], scale=1.0
        )
        nc.scalar.activation(out=rstd[:ts_], in_=rstd[:ts_], func=AF.Exp, scale=-0.5)
        (eng or nc.gpsimd).tensor_scalar(
            out=h_sb[:ts_, t, :],
            in0=x_sb[:ts_, t, :],
            scalar1=mv[:ts_, 0:1],
            scalar2=rstd[:ts_],
            op0=OP.subtract,
            op1=OP.mult,
        )

    def tp_chunk(src_sb, dstT, t, eng=None):
        """[token, 768] chunk -> [768, token] via XBAR DMA (full 128 rows;
        the tail chunk's columns 65: are garbage and never read)."""
        (eng or nc.sync).dma_start_transpose(
            out=dstT[:, t, :, :], in_=src_sb[:, t, :]
        )

    def qk_pair(st_b, hp):
        """qT/kT for head pair hp (feature-major [128, N])."""
        qT = qkp.tile([P, N], BF16, tag="qk")
        kT = qkp.tile([P, N], BF16, tag="qk")
        hT = st_b["hT"]
        for dst, base, col in ((qT, hp * P, hp), (kT, D + hp * P, KD + hp)):
            pq = ps_mm.tile([P, N], F32, tag="ps_mm")
            for k in range(KD):
                for n0, n1 in NSPL:
                    rhs = hT[:, 0:4, k, :] if n0 == 0 else hT[:, 4, k, 0:65]
                    nc.tensor.matmul(
                        pq[:, n0:n1],
                        lhsT=wqkv_sb[:, k, base : base + P],
                        rhs=rhs,
                        start=(k == 0),
                        stop=(k == KD - 1),
                    )
            nc.vector.tensor_scalar_add(
                out=dst, in0=pq, scalar1=bqk_sb[:, col : col + 1]
            )
        st_b["qk"][hp] = (qT, kT)

    def scores_exp(st_b, hp):
        """S^T then exp for both heads of pair hp -> es [js, 2, j, N] bf16."""
        qT, kT = st_b["qk"][hp]
        es = esp.tile([P, 2, NT, N], FP8, tag="es")
        for j in range(NT):
            js = TSZ[j]
            ps_s0 = ps_mm.tile([P, N], F32, tag="ps_mm")
            ps_s1 = ps_mm.tile([P, N], F32, tag="ps_mm")
            ps_s = [ps_s0, ps_s1]
            for n0, n1 in NSPL:
                for sub in range(2):
                    nc.tensor.matmul(
                        ps_s[sub][:js, n0:n1],
                        lhsT=kT[sub * DH : (sub + 1) * DH, TOF[j] : TOF[j] + js],
                        rhs=qT[sub * DH : (sub + 1) * DH, n0:n1],
                        start=True,
                        stop=True,
                        tile_position=(sub * DH, 0),
                    )
            for sub in range(2):
                # bias -2.5 keeps exp within fp8 e4m3 range (softmax is
                # shift-invariant; the factor cancels in the row-sum)
                nc.scalar.activation(
                    out=es[:js, sub, j, :],
                    in_=ps_s[sub][:js, :],
                    func=AF.Exp,
                    bias=nshift_sb[:js],
                    scale=SCALE,
                )
        st_b["es"][hp] = es

    def pv(st_b, hp):
        """o columns for pair hp: PSUM [ts, 2, 70] (64 dh + rowsum col),
        then one reciprocal + one broadcast multiply per token chunk.
        On the last pair, each finished o chunk is transposed immediately."""
        es = st_b["es"][hp]
        v_sb = st_b["v"]
        o_sb = st_b["o"]
        for t in range(NT):
            ts_ = TSZ[t]
            po = ps_pv.tile([P, 2, 70], F32, tag="po")
            for sub in range(2):
                hh = hp * 2 + sub
                for j in range(NT):
                    js = TSZ[j]
                    nc.tensor.matmul(
                        po[:ts_, sub, 0:65],
                        lhsT=es[:js, sub, j, TOF[t] : TOF[t] + ts_],
                        rhs=v_sb[:js, j, hh, :],
                        start=(j == 0),
                        stop=(j == NT - 1),
                    )
            rcp = stat.tile([P, 2], F32, tag="rcp")
            nc.vector.reciprocal(
                out=rcp[:ts_],
                in_=po[:ts_, :, 64:65].rearrange("p s one -> p (s one)"),
            )
            nc.vector.tensor_mul(
                out=o_sb[:ts_, t, hp * P : (hp + 1) * P].rearrange(
                    "p (s e) -> p s e", s=2
                ),
                in0=po[:ts_, :, 0:64],
                in1=_bcast_free(rcp[:ts_, :], DH),
            )
            if hp == KD - 1:
                tp_chunk(o_sb, st_b["oT"], t, eng=(nc.scalar if t % 2 else nc.sync))

    def v_chunk(st_b, t):
        """v[:,t] = h @ Wv + bv (token-major), with an appended ones column."""
        hT = st_b["hT"]
        v_sb = st_b["v"]
        ts_ = TSZ[t]
        pv_ = ps_mm.tile([P, D], F32, tag="ps_mm")
        for k in range(KD):
            for n0, n1 in DSPL:
                nc.tensor.matmul(
                    pv_[:ts_, n0:n1],
                    lhsT=hT[:, t, k, :ts_],
                    rhs=wqkv_sb[:, k, 2 * D + n0 : 2 * D + n1],
                    start=(k == 0),
                    stop=(k == KD - 1),
                )
        nc.vector.tensor_add(
            out=v_sb[:ts_, t, :, 0:DH],
            in0=pv_[:ts_, :].rearrange("p (h e) -> p h e", h=H),
            in1=bv_sb[:ts_, :].rearrange("p (h e) -> p h e", h=H),
        )
        nc.vector.memset(v_sb[:ts_, t, :, DH : DH + 1], 1.0)

    def proj_chunk(st_b, t):
        """x[:,t,:] += o @ Wp + bp (bias via K=1 ones-row matmul)."""
        ts_ = TSZ[t]
        oT, x_sb = st_b["oT"], st_b["x"]
        pp = ps_mm.tile([P, D], F32, tag="ps_mm")
        for k in range(KD):
            for n0, n1 in DSPL:
                nc.tensor.matmul(
                    pp[:ts_, n0:n1],
                    lhsT=oT[:, t, k, :ts_],
                    rhs=wproj_sb[:, k, n0:n1],
                    start=(k == 0),
                    stop=False,
                )
        for n0, n1 in DSPL:
            nc.tensor.matmul(
                pp[:ts_, n0:n1],
                lhsT=ones1[:1, :ts_],
                rhs=bprow[:1, n0:n1],
                start=False,
                stop=True,
            )
        nc.vector.tensor_add(
            out=x_sb[:ts_, t, :], in0=x_sb[:ts_, t, :], in1=pp[:ts_, :]
        )

    def w1_load(f):
        w1t = w1p.tile([P, KD, P], BF16, tag="w1")
        nc.sync.dma_start(
            out=w1t,
            in_=d["w_fc1"][:, f * P : (f + 1) * P].rearrange(
                "(c p) f -> p c f", p=P
            ),
        )
        return w1t

    def fc1(st_b, w1_pre):
        """fc1 (feature-major out) + relu6 -> h1T; wfc1 streamed."""
        h2T = st_b["h2T"]
        h1T = h1p.tile([P, KH, N], BF16, tag="h1")
        npre = len(w1_pre)
        for f in range(KH):
            w1t = w1_pre[f] if f < npre else w1_load(f)
            pf = ps_mm.tile([P, N], F32, tag="ps_mm")
            for k in range(KD):
                for n0, n1 in NSPL:
                    rhs = h2T[:, 0:4, k, :] if n0 == 0 else h2T[:, 4, k, 0:65]
                    nc.tensor.matmul(
                        pf[:, n0:n1],
                        lhsT=w1t[:, k, :],
                        rhs=rhs,
                        start=(k == 0),
                        stop=(k == KD - 1),
                    )
            nc.scalar.activation(
                out=pf, in_=pf, func=AF.Relu, bias=bfc1_sb[:, f : f + 1], scale=1.0
            )
            nc.vector.tensor_scalar_min(out=h1T[:, f, :], in0=pf, scalar1=6.0)
        st_b["h1T"] = h1T

    def fc2_chunk(st_b, t):
        """y[:,t,:] = x2 + h1 @ W2 + b2; then DMA the chunk out."""
        ts_ = TSZ[t]
        h1T, x_sb, b = st_b["h1T"], st_b["x"], st_b["b"]
        pf2 = ps_mm.tile([P, D], F32, tag="ps_mm")
        for k in range(KH):
            for n0, n1 in DSPL:
                nc.tensor.matmul(
                    pf2[:ts_, n0:n1],
                    lhsT=h1T[:, k, TOF[t] : TOF[t] + ts_],
                    rhs=wfc2_sb[:, k, n0:n1],
                    start=(k == 0),
                    stop=False,
                )
        for n0, n1 in DSPL:
            nc.tensor.matmul(
                pf2[:ts_, n0:n1],
                lhsT=ones1[:1, :ts_],
                rhs=bf2row[:1, n0:n1],
                start=False,
                stop=True,
            )
        nc.vector.tensor_add(
            out=x_sb[:ts_, t, :], in0=x_sb[:ts_, t, :], in1=pf2[:ts_, :]
        )
        nc.sync.dma_start(
            out=y_d[b, TOF[t] : TOF[t] + ts_, :], in_=x_sb[:ts_, t, :]
        )

    loop_reps = d.get("_reps", 1)

    def emit_all_fixed(n_passes=1):
        state = {}

        def new_state(i):
            state[i] = {"b": i % BPC, "qk": {}, "es": {}}
            return state[i]

        nb = n_passes * BPC
        s0 = new_state(0)
        s0["x"] = load_x(0)
        h0 = hbf.tile([P, NT, D], BF16, tag="hbf")
        hT0 = t6.tile([P, NT, KD, P], BF16, tag="t6")
        for t in range(NT):
            ln_chunk(s0["x"], h0, t)
            tp_chunk(h0, hT0, t)
        s0["hT"] = hT0
        v0 = vvp.tile([P, NT, H, DH + 1], FP8, tag="vv")
        s0["v"] = v0
        for t in range(NT):
            v_chunk(s0, t)
        for bi in range(nb):
            cur = state[bi]
            prv = state.get(bi - 1)
            nxt = new_state(bi + 1) if bi + 1 < nb else None

            # ---- attention(cur) ⊗ fc2(prv) ⊗ load/LN1(nxt) ----
            o_cur = hbf.tile([P, NT, D], BF16, tag="hbf")
            cur["o"] = o_cur
            oT = t6.tile([P, NT, KD, P], BF16, tag="t6")
            cur["oT"] = oT
            hn = None
            if nxt is not None:
                xn = xp.tile([P, NT, D], BF16, tag="x")
                nxt["x"] = xn
                hn = hbf.tile([P, NT, D], BF16, tag="hbf")
                hTn = t6.tile([P, NT, KD, P], BF16, tag="t6")
                nxt["hT"] = hTn
                vn = vvp.tile([P, NT, H, DH + 1], FP8, tag="vv")
                nxt["v"] = vn
            qk_pair(cur, 0)
            qk_pair(cur, 1)
            scores_exp(cur, 0)
            for hp in range(KD):
                if hp + 2 < KD:
                    qk_pair(cur, hp + 2)
                if hp + 1 < KD:
                    scores_exp(cur, hp + 1)
                if prv is not None and hp < NT:
                    fc2_chunk(prv, hp)
                if nxt is not None and hp < NT:
                    nc.sync.dma_start(
                        out=xn[: TSZ[hp], hp, :],
                        in_=x_d[nxt["b"], TOF[hp] : TOF[hp] + TSZ[hp], :],
                    )
                if nxt is not None and hp >= 1:
                    ln_chunk(xn, hn, hp - 1)
                    tp_chunk(hn, hTn, hp - 1)
                pv(cur, hp)
            if nxt is not None:
                ln_chunk(xn, hn, NT - 1)
                tp_chunk(hn, hTn, NT - 1)
            if prv is not None:
                del state[bi - 1]

            # ---- proj(cur) with the LN2(cur) chain per chunk ----
            h2 = hbf.tile([P, NT, D], BF16, tag="hbf")
            h2T = t6.tile([P, NT, KD, P], BF16, tag="t6")
            cur["h2T"] = h2T
            w1_pre = []
            for t in range(NT):
                proj_chunk(cur, t)
                ln_chunk(cur["x"], h2, t)
                tp_chunk(h2, h2T, t)
                if t >= 2:
                    w1_pre.append(w1_load(t - 2))
            # ---- v(nxt): PE cover for the LN2 chain tail ----
            if nxt is not None:
                for t in range(NT):
                    v_chunk(nxt, t)

            fc1(cur, w1_pre)

            if bi == nb - 1:
                for t in range(NT):
                    fc2_chunk(cur, t)

    if loop_reps > 1 and d.get("_unroll"):
        for _ in range(loop_reps):
            emit_all_fixed(1)
    elif loop_reps > 1 and loop_reps % 2 == 0:
        # amortize the For_i all-engine barrier over 2 connected passes
        with tc.For_i(0, loop_reps // 2, 1):
            emit_all_fixed(2)
    elif loop_reps > 1:
        with tc.For_i(0, loop_reps, 1):
            emit_all_fixed(1)
    else:
        emit_all_fixed(1)


def build_nc(reps=1, unroll=False):
    nc = bacc.Bacc("TRN2", target_bir_lowering=False, debug=False)
    d = {
        "_reps": reps,
        "_unroll": unroll,
        "x": nc.dram_tensor("x", [BPC, N, D], BF16, kind="ExternalInput"),
        "w_qkv": nc.dram_tensor("w_qkv", [D, 3 * D], BF16, kind="ExternalInput"),
        "b_qk": nc.dram_tensor("b_qk", [P, 12], F32, kind="ExternalInput"),
        "b_v": nc.dram_tensor("b_v", [D], BF16, kind="ExternalInput"),
        "w_proj": nc.dram_tensor("w_proj", [D, D], BF16, kind="ExternalInput"),
        "b_proj": nc.dram_tensor("b_proj", [D], BF16, kind="ExternalInput"),
        "w_fc1": nc.dram_tensor("w_fc1", [D, HID], BF16, kind="ExternalInput"),
        "b_fc1": nc.dram_tensor("b_fc1", [P, KH], F32, kind="ExternalInput"),
        "w_fc2": nc.dram_tensor("w_fc2", [HID, D], BF16, kind="ExternalInput"),
        "b_fc2": nc.dram_tensor("b_fc2", [D], BF16, kind="ExternalInput"),
        "y": nc.dram_tensor("y", [BPC, N, D], BF16, kind="ExternalOutput"),
    }
    with tile.TileContext(nc) as tc:
        with ExitStack() as ctx:
            _body(ctx, tc, d)
    nc.compile()
    return nc


def host_inputs(inputs):
    """Fold LN affine params into weights; cast matmul operands to bf16."""
    bf = ml_dtypes.bfloat16
    f32 = np.float32
    g1 = np.asarray(inputs["ln1_g"], f32)
    b1 = np.asarray(inputs["ln1_b"], f32)
    g2 = np.asarray(inputs["ln2_g"], f32)
    b2 = np.asarray(inputs["ln2_b"], f32)
    w_qkv = np.asarray(inputs["w_qkv"], f32)
    w_fc1 = np.asarray(inputs["w_fc1"], f32)
    b_qkv_eff = np.asarray(inputs["b_qkv"], f32) + b1 @ w_qkv
    b_fc1_eff = np.asarray(inputs["b_fc1"], f32) + b2 @ w_fc1
    return {
        "w_qkv": (g1[:, None] * w_qkv).astype(bf),
        "b_qk": np.ascontiguousarray(
            b_qkv_eff[: 2 * D].reshape(12, P).T
        ).astype(f32),
        "b_v": b_qkv_eff[2 * D :].astype(bf),
        "w_proj": np.asarray(inputs["w_proj"], f32).astype(bf),
        "b_proj": np.asarray(inputs["b_proj"], f32).astype(bf),
        "w_fc1": (g2[:, None] * w_fc1).astype(bf),
        "b_fc1": np.ascontiguousarray(b_fc1_eff.reshape(KH, P).T).astype(f32),
        "w_fc2": np.asarray(inputs["w_fc2"], f32).astype(bf),
        "b_fc2": np.asarray(inputs["b_fc2"], f32).astype(bf),
    }


_CACHE = {}


def get_runner(reps=1, unroll=False):
    """Build (once) the bass module and a persistent 8-core PJRT runner."""
    key = ("runner", reps, unroll)
    if key in _CACHE:
        return _CACHE[key]

    import jax
    from jax.sharding import Mesh, PartitionSpec
    from jax.experimental.shard_map import shard_map
    from concourse import bass2jax, mybir as mb

    bass2jax.install_neuronx_cc_hook()
    nc = build_nc(reps=reps, unroll=unroll)

    partition_name = nc.partition_id_tensor.name if nc.partition_id_tensor else None
    in_names, out_names, out_avals = [], [], []
    for alloc in nc.m.functions[0].allocations:
        if not isinstance(alloc, mb.MemoryLocationSet):
            continue
        name = alloc.memorylocations[0].name
        if alloc.kind == "ExternalInput":
            if name != partition_name:
                in_names.append(name)
        elif alloc.kind == "ExternalOutput":
            out_names.append(name)
            out_avals.append(
                jax.core.ShapedArray(tuple(alloc.tensor_shape), mb.dt.np(alloc.dtype))
            )
    n_params = len(in_names)
    n_outs = len(out_names)
    all_names = in_names + out_names
    if partition_name is not None:
        all_names = all_names + [partition_name]

    def _run_body(*args):
        operands = list(args)
        if partition_name is not None:
            operands.append(bass2jax.partition_id_tensor())
        return tuple(
            bass2jax._bass_exec_p.bind(
                *operands,
                out_avals=tuple(out_avals),
                in_names=tuple(all_names),
                out_names=tuple(out_names),
                lowering_input_output_aliases=(),
                sim_require_finite=True,
                sim_require_nnan=True,
                nc=nc,
            )
        )

    devices = jax.devices()[:NCORES]
    mesh = Mesh(np.asarray(devices), ("core",))
    donate = tuple(range(n_params, n_params + n_outs))
    sharded = jax.jit(
        shard_map(
            _run_body,
            mesh=mesh,
            in_specs=(PartitionSpec("core"),) * (n_params + n_outs),
            out_specs=(PartitionSpec("core"),) * n_outs,
            check_rep=False,
        ),
        donate_argnums=donate,
        keep_unused=True,
    )

    def run(in_maps, timeit=False):
        concat_in = [
            np.concatenate([np.asarray(m[name]) for m in in_maps], axis=0)
            for name in in_names
        ]
        concat_in = [jax.device_put(a) for a in concat_in]
        zeros = [
            jax.device_put(
                np.zeros((NCORES * av.shape[0], *av.shape[1:]), av.dtype)
            )
            for av in out_avals
        ]
        for a in concat_in + zeros:
            a.block_until_ready()
        t0 = time.monotonic()
        out_arrs = sharded(*concat_in, *zeros)
        for o in out_arrs:
            o.block_until_ready()
        dt = time.monotonic() - t0
        res = [
            {
                name: np.asarray(out_arrs[i]).reshape(
                    NCORES, *out_avals[i].shape
                )[c]
                for i, name in enumerate(out_names)
            }
            for c in range(NCORES)
        ]
        if timeit:
            return res, dt
        return res

    _CACHE[key] = run
    return run


def make_in_maps(inputs):
    x = np.asarray(inputs["x"], np.float32).astype(ml_dtypes.bfloat16)
    shared = host_inputs(inputs)
    return [
        {"x": np.ascontiguousarray(x[c * BPC : (c + 1) * BPC]), **shared}
        for c in range(NCORES)
    ]


def kernel(**inputs):
    run = get_runner()
    in_maps = make_in_maps(inputs)
    res = run(in_maps)
    y = np.concatenate([np.asarray(r["y"]) for r in res], axis=0)
    return y.astype(np.float32)



# revision 9
# speedup vs baseline: 1.1149x; 1.1149x over previous
"""Fused ViT-style transformer block on 8 TRN2 NeuronCores (pipelined v2).

Sharding: data-parallel over batch (32 batches -> 4 per core). Each core runs
the full block (LN1 -> QKV -> attention -> proj -> residual -> LN2 -> MLP ->
residual) on its 4 batches. No collectives.

v2 structure: the 4 per-core batches are software-pipelined so the PE never
waits on the LayerNorm/softmax chains:
  - fc2 of batch b-1 is interleaved into batch b's attention loop (PE work
    under the ACT-bound exp stream).
  - LN1 of batch b+1 runs during batch b's o-transpose/proj window.
  - v-matmuls of batch b+1 fill the LN2(b) stats/normalize/transpose chain.
Other changes vs v1:
  - LN rstd = exp(-0.5*ln(var+eps)) so the single natural_log_exp ACT table
    serves LN + softmax + relu with zero table switches.
  - proj/fc2 biases enter PSUM as K=1 ones-row matmuls (Pool engine freed).
  - All transposes ride the DMA xbar (tail chunk padded to 128 rows; the
    garbage columns are never read), so the PE matmul stream is never
    interrupted and no PSUM bank is spent on transposes.
  - PV accumulates both heads of a pair into one PSUM bank with the softmax
    row-sum column; normalize is one reciprocal + one broadcast multiply.
"""

import time

import numpy as np
import ml_dtypes
from contextlib import ExitStack

import concourse.bass as bass
import concourse.bacc as bacc
import concourse.tile as tile
from concourse import mybir

# Constrain the ACT table-set chooser to natural_log_exp_and_others, which
# covers every activation this kernel uses (ln, exp, relu, copy, identity).
# The default chooser picks the first set containing each function, which
# thrashes table loads (~2.7us each) between the LN rstd chain and softmax.
# Set indices are preserved (other sets are emptied, not removed).
_NLE = "natural_log_exp_and_others"
_gat_orig = bacc.get_activation_tables


def _gat_nle_only(arch):
    t = _gat_orig(arch)
    assert _NLE in t
    return {k: (v if k == _NLE else set()) for k, v in t.items()}


bacc.get_activation_tables = _gat_nle_only

F32 = mybir.dt.float32
BF16 = mybir.dt.bfloat16
FP8 = mybir.dt.float8e4
AF = mybir.ActivationFunctionType
OP = mybir.AluOpType

B, N, D, H = 32, 577, 768, 12
DH = D // H            # 64
HID = 4 * D            # 3072
NCORES = 8
BPC = B // NCORES      # batches per core
P = 128
KD = D // P            # 6
KH = HID // P          # 24
NC1 = KH // 4          # wfc1 stream chunks (4 column tiles each)
NT = 5                 # token chunks per batch: 4x128 + 65
TSZ = [128, 128, 128, 128, 65]
TOF = [0, 128, 256, 384, 512]
EPS = 1e-5
SCALE = DH ** -0.5


def _bcast(ap_1d, p=P):
    """AP that reads a 1-D dram tensor broadcast across p partitions."""
    return bass.AP(
        tensor=ap_1d.tensor, offset=ap_1d.offset, ap=[[0, p]] + list(ap_1d.ap)
    )


def _bcast_free(ap, n):
    """AP with an appended stride-0 free dim of length n (broadcast)."""
    return bass.AP(tensor=ap.tensor, offset=ap.offset, ap=list(ap.ap) + [[0, n]])


def _body(ctx, tc, d):
    nc = tc.nc

    const = ctx.enter_context(tc.tile_pool(name="const", bufs=1))
    xp = ctx.enter_context(tc.tile_pool(name="xp", bufs=3))
    hbf = ctx.enter_context(tc.tile_pool(name="hbf", bufs=2))
    t6 = ctx.enter_context(tc.tile_pool(name="t6", bufs=3))
    vvp = ctx.enter_context(tc.tile_pool(name="vv", bufs=1))
    qkp = ctx.enter_context(tc.tile_pool(name="qk", bufs=4))
    esp = ctx.enter_context(tc.tile_pool(name="es", bufs=2))
    h1p = ctx.enter_context(tc.tile_pool(name="h1", bufs=1))
    w1p = ctx.enter_context(tc.tile_pool(name="w1", bufs=3))
    stat = ctx.enter_context(tc.tile_pool(name="stat", bufs=6))
    ps_mm = ctx.enter_context(tc.tile_pool(name="ps_mm", bufs=3, space="PSUM"))
    ps_pv = ctx.enter_context(tc.tile_pool(name="ps_pv", bufs=2, space="PSUM"))

    # ---- one-time constants ----
    eps_sb = const.tile([P, 1], F32)
    nc.vector.memset(eps_sb, EPS)
    nshift_sb = const.tile([P, 1], F32)
    nc.vector.memset(nshift_sb, -2.5)
    ones1 = const.tile([1, P], BF16)
    nc.vector.memset(ones1, 1.0)

    wqkv_sb = const.tile([P, KD, 3 * D], BF16)
    # v-columns first: the v matmuls are the first consumers of wqkv
    nc.gpsimd.dma_start(
        out=wqkv_sb[:, :, 2 * D :],
        in_=d["w_qkv"][:, 2 * D :].rearrange("(c p) f -> p c f", p=P),
    )
    nc.gpsimd.dma_start(
        out=wqkv_sb[:, :, : 2 * D],
        in_=d["w_qkv"][:, : 2 * D].rearrange("(c p) f -> p c f", p=P),
    )
    # wproj/wfc2 tiles are DMA'd after batch 0's v-phase so the Pool queue
    # isn't hogged at startup
    wproj_sb = const.tile([P, KD, D], BF16)
    nc.gpsimd.dma_start(
        out=wproj_sb, in_=d["w_proj"][:].rearrange("(c p) f -> p c f", p=P)
    )
    wfc2_sb = const.tile([P, KH, D], BF16)
    nc.gpsimd.dma_start(
        out=wfc2_sb, in_=d["w_fc2"][:].rearrange("(c p) f -> p c f", p=P)
    )
    bqk_sb = const.tile([P, 12], F32)
    nc.sync.dma_start(out=bqk_sb, in_=d["b_qk"][:])
    bfc1_sb = const.tile([P, KH], F32)
    nc.sync.dma_start(out=bfc1_sb, in_=d["b_fc1"][:])
    bv_sb = const.tile([P, D], BF16)
    nc.sync.dma_start(out=bv_sb, in_=_bcast(d["b_v"][:]))
    bprow = const.tile([1, D], BF16)
    nc.sync.dma_start(out=bprow, in_=_bcast(d["b_proj"][:], p=1))
    bf2row = const.tile([1, D], BF16)
    nc.sync.dma_start(out=bf2row, in_=_bcast(d["b_fc2"][:], p=1))

    x_d, y_d = d["x"], d["y"]

    NSPL = ((0, 512), (512, N))    # token-free splits (qk/scores/fc1 rhs)
    DSPL = ((0, 512), (512, D))    # feature-free splits (v/proj/fc2 out)

    def load_x(b):
        x_sb = xp.tile([P, NT, D], BF16, tag="x")
        for t in range(NT):
            nc.sync.dma_start(
                out=x_sb[: TSZ[t], t, :],
                in_=x_d[b, TOF[t] : TOF[t] + TSZ[t], :],
            )
        return x_sb

    def ln_chunk(x_sb, h_sb, t, eng=None):
        """h[:,t,:] = (x - mu) * rsqrt(var + eps); rstd via exp(-0.5*ln())."""
        ts_ = TSZ[t]
        st = stat.tile([P, 2, 6], F32, tag="bnst")
        mv = stat.tile([P, 2], F32, tag="bnmv")
        xin = x_sb[:ts_, t, :].rearrange("p (s q) -> p s q", s=2)
        for s in range(2):
            nc.vector.bn_stats(out=st[:ts_, s, :], in_=xin[:, s, :])
        nc.vector.bn_aggr(out=mv[:ts_], in_=st[:ts_])
        rstd = stat.tile([P, 1], F32, tag="rstd")
        nc.scalar.activation(
            out=rstd[:ts_], in_=mv[:ts_, 1:2], func=AF.Ln, bias=eps_sb[:ts_], scale=1.0
        )
        nc.scalar.activation(out=rstd[:ts_], in_=rstd[:ts_], func=AF.Exp, scale=-0.5)
        (eng or nc.gpsimd).tensor_scalar(
            out=h_sb[:ts_, t, :],
            in0=x_sb[:ts_, t, :],
            scalar1=mv[:ts_, 0:1],
            scalar2=rstd[:ts_],
            op0=OP.subtract,
            op1=OP.mult,
        )

    def tp_chunk(src_sb, dstT, t, eng=None):
        """[token, 768] chunk -> [768, token] via XBAR DMA (full 128 rows;
        the tail chunk's columns 65: are garbage and never read)."""
        (eng or nc.sync).dma_start_transpose(
            out=dstT[:, t, :, :], in_=src_sb[:, t, :]
        )

    def qk_pair(st_b, hp):
        """qT/kT for head pair hp (feature-major [128, N])."""
        qT = qkp.tile([P, N], BF16, tag="qk")
        kT = qkp.tile([P, N], BF16, tag="qk")
        hT = st_b["hT"]
        for dst, base, col in ((qT, hp * P, hp), (kT, D + hp * P, KD + hp)):
            pq = ps_mm.tile([P, N], F32, tag="ps_mm")
            for k in range(KD):
                for n0, n1 in NSPL:
                    rhs = hT[:, 0:4, k, :] if n0 == 0 else hT[:, 4, k, 0:65]
                    nc.tensor.matmul(
                        pq[:, n0:n1],
                        lhsT=wqkv_sb[:, k, base : base + P],
                        rhs=rhs,
                        start=(k == 0),
                        stop=(k == KD - 1),
                    )
            nc.vector.tensor_scalar_add(
                out=dst, in0=pq, scalar1=bqk_sb[:, col : col + 1]
            )
        st_b["qk"][hp] = (qT, kT)

    def scores_exp(st_b, hp):
        """S^T then exp for both heads of pair hp -> es [js, 2, j, N] bf16."""
        qT, kT = st_b["qk"][hp]
        es = esp.tile([P, 2, NT, N], FP8, tag="es")
        for j in range(NT):
            js = TSZ[j]
            ps_s0 = ps_mm.tile([P, N], F32, tag="ps_mm")
            ps_s1 = ps_mm.tile([P, N], F32, tag="ps_mm")
            ps_s = [ps_s0, ps_s1]
            for n0, n1 in NSPL:
                for sub in range(2):
                    nc.tensor.matmul(
                        ps_s[sub][:js, n0:n1],
                        lhsT=kT[sub * DH : (sub + 1) * DH, TOF[j] : TOF[j] + js],
                        rhs=qT[sub * DH : (sub + 1) * DH, n0:n1],
                        start=True,
                        stop=True,
                        tile_position=(sub * DH, 0),
                    )
            for sub in range(2):
                # bias -2.5 keeps exp within fp8 e4m3 range (softmax is
                # shift-invariant; the factor cancels in the row-sum)
                nc.scalar.activation(
                    out=es[:js, sub, j, :],
                    in_=ps_s[sub][:js, :],
                    func=AF.Exp,
                    bias=nshift_sb[:js],
                    scale=SCALE,
                )
        st_b["es"][hp] = es

    def pv(st_b, hp):
        """o columns for pair hp: PSUM [ts, 2, 70] (64 dh + rowsum col),
        then one reciprocal + one broadcast multiply per token chunk.
        On the last pair, each finished o chunk is transposed immediately."""
        es = st_b["es"][hp]
        v_sb = st_b["v"]
        o_sb = st_b["o"]
        for t in range(NT):
            ts_ = TSZ[t]
            po = ps_pv.tile([P, 2, 70], F32, tag="po")
            for sub in range(2):
                hh = hp * 2 + sub
                for j in range(NT):
                    js = TSZ[j]
                    nc.tensor.matmul(
                        po[:ts_, sub, 0:65],
                        lhsT=es[:js, sub, j, TOF[t] : TOF[t] + ts_],
                        rhs=v_sb[:js, j, hh, :],
                        start=(j == 0),
                        stop=(j == NT - 1),
                    )
            rcp = stat.tile([P, 2], F32, tag="rcp")
            nc.vector.reciprocal(
                out=rcp[:ts_],
                in_=po[:ts_, :, 64:65].rearrange("p s one -> p (s one)"),
            )
            nc.vector.tensor_mul(
                out=o_sb[:ts_, t, hp * P : (hp + 1) * P].rearrange(
                    "p (s e) -> p s e", s=2
                ),
                in0=po[:ts_, :, 0:64],
                in1=_bcast_free(rcp[:ts_, :], DH),
            )
            if hp == KD - 1:
                tp_chunk(o_sb, st_b["oT"], t, eng=(nc.scalar if t % 2 else nc.sync))

    def v_chunk(st_b, t):
        """v[:,t] = h @ Wv + bv (token-major), with an appended ones column."""
        hT = st_b["hT"]
        v_sb = st_b["v"]
        ts_ = TSZ[t]
        pv_ = ps_mm.tile([P, D], F32, tag="ps_mm")
        for k in range(KD):
            for n0, n1 in DSPL:
                nc.tensor.matmul(
                    pv_[:ts_, n0:n1],
                    lhsT=hT[:, t, k, :ts_],
                    rhs=wqkv_sb[:, k, 2 * D + n0 : 2 * D + n1],
                    start=(k == 0),
                    stop=(k == KD - 1),
                )
        nc.vector.tensor_add(
            out=v_sb[:ts_, t, :, 0:DH],
            in0=pv_[:ts_, :].rearrange("p (h e) -> p h e", h=H),
            in1=bv_sb[:ts_, :].rearrange("p (h e) -> p h e", h=H),
        )
        nc.vector.memset(v_sb[:ts_, t, :, DH : DH + 1], 1.0)

    def proj_chunk(st_b, t):
        """x[:,t,:] += o @ Wp + bp (bias via K=1 ones-row matmul)."""
        ts_ = TSZ[t]
        oT, x_sb = st_b["oT"], st_b["x"]
        pp = ps_mm.tile([P, D], F32, tag="ps_mm")
        for k in range(KD):
            for n0, n1 in DSPL:
                nc.tensor.matmul(
                    pp[:ts_, n0:n1],
                    lhsT=oT[:, t, k, :ts_],
                    rhs=wproj_sb[:, k, n0:n1],
                    start=(k == 0),
                    stop=False,
                )
        for n0, n1 in DSPL:
            nc.tensor.matmul(
                pp[:ts_, n0:n1],
                lhsT=ones1[:1, :ts_],
                rhs=bprow[:1, n0:n1],
                start=False,
                stop=True,
            )
        nc.vector.tensor_add(
            out=x_sb[:ts_, t, :], in0=x_sb[:ts_, t, :], in1=pp[:ts_, :]
        )

    def w1_load(c0):
        """One 4-column-tile chunk of wfc1: [P, KD, 512], contiguous 6KB per
        partition in DRAM (host pre-shuffled), issued on the ACT queue so the
        saturated SP queue never sees it."""
        w1t = w1p.tile([P, KD, 4 * P], BF16, tag="w1")
        nc.scalar.dma_start(out=w1t, in_=d["w_fc1"][:, c0])
        return w1t

    def fc1(st_b, w1_pre):
        """fc1 (feature-major out) + relu6 -> h1T; wfc1 streamed in chunks."""
        h2T = st_b["h2T"]
        h1T = h1p.tile([P, KH, N], BF16, tag="h1")
        w1c = list(w1_pre)
        for f in range(KH):
            c0, r = divmod(f, 4)
            if r == 0 and len(w1c) < NC1:
                w1c.append(w1_load(len(w1c)))
            w1t = w1c[c0]
            pf = ps_mm.tile([P, N], F32, tag="ps_mm")
            for k in range(KD):
                for n0, n1 in NSPL:
                    rhs = h2T[:, 0:4, k, :] if n0 == 0 else h2T[:, 4, k, 0:65]
                    nc.tensor.matmul(
                        pf[:, n0:n1],
                        lhsT=w1t[:, k, r * P : (r + 1) * P],
                        rhs=rhs,
                        start=(k == 0),
                        stop=(k == KD - 1),
                    )
            nc.scalar.activation(
                out=pf, in_=pf, func=AF.Relu, bias=bfc1_sb[:, f : f + 1], scale=1.0
            )
            nc.vector.tensor_scalar_min(out=h1T[:, f, :], in0=pf, scalar1=6.0)
        st_b["h1T"] = h1T

    def fc2_chunk(st_b, t):
        """y[:,t,:] = x2 + h1 @ W2 + b2; then DMA the chunk out."""
        ts_ = TSZ[t]
        h1T, x_sb, b = st_b["h1T"], st_b["x"], st_b["b"]
        pf2 = ps_mm.tile([P, D], F32, tag="ps_mm")
        for k in range(KH):
            for n0, n1 in DSPL:
                nc.tensor.matmul(
                    pf2[:ts_, n0:n1],
                    lhsT=h1T[:, k, TOF[t] : TOF[t] + ts_],
                    rhs=wfc2_sb[:, k, n0:n1],
                    start=(k == 0),
                    stop=False,
                )
        for n0, n1 in DSPL:
            nc.tensor.matmul(
                pf2[:ts_, n0:n1],
                lhsT=ones1[:1, :ts_],
                rhs=bf2row[:1, n0:n1],
                start=False,
                stop=True,
            )
        nc.vector.tensor_add(
            out=x_sb[:ts_, t, :], in0=x_sb[:ts_, t, :], in1=pf2[:ts_, :]
        )
        nc.gpsimd.dma_start(
            out=y_d[b, TOF[t] : TOF[t] + ts_, :], in_=x_sb[:ts_, t, :]
        )

    loop_reps = d.get("_reps", 1)

    def emit_all_fixed(n_passes=1):
        state = {}

        def new_state(i):
            state[i] = {"b": i % BPC, "qk": {}, "es": {}}
            return state[i]

        nb = n_passes * BPC
        s0 = new_state(0)
        s0["x"] = load_x(0)
        h0 = hbf.tile([P, NT, D], BF16, tag="hbf")
        hT0 = t6.tile([P, NT, KD, P], BF16, tag="t6")
        for t in range(NT):
            ln_chunk(s0["x"], h0, t)
            tp_chunk(h0, hT0, t)
        s0["hT"] = hT0
        v0 = vvp.tile([P, NT, H, DH + 1], FP8, tag="vv")
        s0["v"] = v0
        for t in range(NT):
            v_chunk(s0, t)
        for bi in range(nb):
            cur = state[bi]
            prv = state.get(bi - 1)
            nxt = new_state(bi + 1) if bi + 1 < nb else None

            # ---- attention(cur) ⊗ fc2(prv) ⊗ load/LN1(nxt) ----
            o_cur = hbf.tile([P, NT, D], BF16, tag="hbf")
            cur["o"] = o_cur
            oT = t6.tile([P, NT, KD, P], BF16, tag="t6")
            cur["oT"] = oT
            hn = None
            if nxt is not None:
                xn = xp.tile([P, NT, D], BF16, tag="x")
                nxt["x"] = xn
                hn = hbf.tile([P, NT, D], BF16, tag="hbf")
                hTn = t6.tile([P, NT, KD, P], BF16, tag="t6")
                nxt["hT"] = hTn
                vn = vvp.tile([P, NT, H, DH + 1], FP8, tag="vv")
                nxt["v"] = vn
            qk_pair(cur, 0)
            qk_pair(cur, 1)
            scores_exp(cur, 0)
            for hp in range(KD):
                if hp + 2 < KD:
                    qk_pair(cur, hp + 2)
                if hp + 1 < KD:
                    scores_exp(cur, hp + 1)
                if prv is not None and hp < NT:
                    fc2_chunk(prv, hp)
                if nxt is not None and hp < NT:
                    nc.sync.dma_start(
                        out=xn[: TSZ[hp], hp, :],
                        in_=x_d[nxt["b"], TOF[hp] : TOF[hp] + TSZ[hp], :],
                    )
                if nxt is not None and hp >= 1:
                    ln_chunk(xn, hn, hp - 1)
                    tp_chunk(hn, hTn, hp - 1)
                pv(cur, hp)
            if nxt is not None:
                ln_chunk(xn, hn, NT - 1)
                tp_chunk(hn, hTn, NT - 1)
            if prv is not None:
                del state[bi - 1]

            # ---- proj(cur) with the LN2(cur) chain per chunk ----
            h2 = hbf.tile([P, NT, D], BF16, tag="hbf")
            h2T = t6.tile([P, NT, KD, P], BF16, tag="t6")
            cur["h2T"] = h2T
            w1_pre = []
            for t in range(NT):
                proj_chunk(cur, t)
                ln_chunk(cur["x"], h2, t)
                tp_chunk(h2, h2T, t)
                if t >= 3:
                    w1_pre.append(w1_load(t - 3))
            # ---- v(nxt): PE cover for the LN2 chain tail ----
            if nxt is not None:
                for t in range(NT):
                    v_chunk(nxt, t)

            fc1(cur, w1_pre)

            if bi == nb - 1:
                for t in range(NT):
                    fc2_chunk(cur, t)

    if loop_reps > 1 and d.get("_unroll"):
        for _ in range(loop_reps):
            emit_all_fixed(1)
    elif loop_reps > 1 and loop_reps % 4 == 0:
        # amortize the For_i all-engine barrier over 4 connected passes
        with tc.For_i(0, loop_reps // 4, 1):
            emit_all_fixed(4)
    elif loop_reps > 1 and loop_reps % 2 == 0:
        # amortize the For_i all-engine barrier over 2 connected passes
        with tc.For_i(0, loop_reps // 2, 1):
            emit_all_fixed(2)
    elif loop_reps > 1:
        with tc.For_i(0, loop_reps, 1):
            emit_all_fixed(1)
    else:
        emit_all_fixed(1)


def build_nc(reps=1, unroll=False):
    nc = bacc.Bacc("TRN2", target_bir_lowering=False, debug=False)
    d = {
        "_reps": reps,
        "_unroll": unroll,
        "x": nc.dram_tensor("x", [BPC, N, D], BF16, kind="ExternalInput"),
        "w_qkv": nc.dram_tensor("w_qkv", [D, 3 * D], BF16, kind="ExternalInput"),
        "b_qk": nc.dram_tensor("b_qk", [P, 12], F32, kind="ExternalInput"),
        "b_v": nc.dram_tensor("b_v", [D], BF16, kind="ExternalInput"),
        "w_proj": nc.dram_tensor("w_proj", [D, D], BF16, kind="ExternalInput"),
        "b_proj": nc.dram_tensor("b_proj", [D], BF16, kind="ExternalInput"),
        "w_fc1": nc.dram_tensor(
            "w_fc1", [P, NC1, KD, 4 * P], BF16, kind="ExternalInput"
        ),
        "b_fc1": nc.dram_tensor("b_fc1", [P, KH], F32, kind="ExternalInput"),
        "w_fc2": nc.dram_tensor("w_fc2", [HID, D], BF16, kind="ExternalInput"),
        "b_fc2": nc.dram_tensor("b_fc2", [D], BF16, kind="ExternalInput"),
        "y": nc.dram_tensor("y", [BPC, N, D], BF16, kind="ExternalOutput"),
    }
    with tile.TileContext(nc) as tc:
        with ExitStack() as ctx:
            _body(ctx, tc, d)
    nc.compile()
    return nc


def host_inputs(inputs):
    """Fold LN affine params into weights; cast matmul operands to bf16."""
    bf = ml_dtypes.bfloat16
    f32 = np.float32
    g1 = np.asarray(inputs["ln1_g"], f32)
    b1 = np.asarray(inputs["ln1_b"], f32)
    g2 = np.asarray(inputs["ln2_g"], f32)
    b2 = np.asarray(inputs["ln2_b"], f32)
    w_qkv = np.asarray(inputs["w_qkv"], f32)
    w_fc1 = np.asarray(inputs["w_fc1"], f32)
    b_qkv_eff = np.asarray(inputs["b_qkv"], f32) + b1 @ w_qkv
    b_fc1_eff = np.asarray(inputs["b_fc1"], f32) + b2 @ w_fc1
    return {
        "w_qkv": (g1[:, None] * w_qkv).astype(bf),
        "b_qk": np.ascontiguousarray(
            b_qkv_eff[: 2 * D].reshape(12, P).T
        ).astype(f32),
        "b_v": b_qkv_eff[2 * D :].astype(bf),
        "w_proj": np.asarray(inputs["w_proj"], f32).astype(bf),
        "b_proj": np.asarray(inputs["b_proj"], f32).astype(bf),
        # [p, chunk, c, fi]: each [c, fi] plane contiguous per partition so a
        # chunk load is 128 descriptors of 6KB instead of 768 of 256B
        "w_fc1": np.ascontiguousarray(
            (g2[:, None] * w_fc1)
            .reshape(KD, P, NC1, 4 * P)
            .transpose(1, 2, 0, 3)
        ).astype(bf),
        "b_fc1": np.ascontiguousarray(b_fc1_eff.reshape(KH, P).T).astype(f32),
        "w_fc2": np.asarray(inputs["w_fc2"], f32).astype(bf),
        "b_fc2": np.asarray(inputs["b_fc2"], f32).astype(bf),
    }


_CACHE = {}


def get_runner(reps=1, unroll=False):
    """Build (once) the bass module and a persistent 8-core PJRT runner."""
    key = ("runner", reps, unroll)
    if key in _CACHE:
        return _CACHE[key]

    import jax
    from jax.sharding import Mesh, PartitionSpec
    from jax.experimental.shard_map import shard_map
    from concourse import bass2jax, mybir as mb

    bass2jax.install_neuronx_cc_hook()
    nc = build_nc(reps=reps, unroll=unroll)

    partition_name = nc.partition_id_tensor.name if nc.partition_id_tensor else None
    in_names, out_names, out_avals = [], [], []
    for alloc in nc.m.functions[0].allocations:
        if not isinstance(alloc, mb.MemoryLocationSet):
            continue
        name = alloc.memorylocations[0].name
        if alloc.kind == "ExternalInput":
            if name != partition_name:
                in_names.append(name)
        elif alloc.kind == "ExternalOutput":
            out_names.append(name)
            out_avals.append(
                jax.core.ShapedArray(tuple(alloc.tensor_shape), mb.dt.np(alloc.dtype))
            )
    n_params = len(in_names)
    n_outs = len(out_names)
    all_names = in_names + out_names
    if partition_name is not None:
        all_names = all_names + [partition_name]

    def _run_body(*args):
        operands = list(args)
        if partition_name is not None:
            operands.append(bass2jax.partition_id_tensor())
        return tuple(
            bass2jax._bass_exec_p.bind(
                *operands,
                out_avals=tuple(out_avals),
                in_names=tuple(all_names),
                out_names=tuple(out_names),
                lowering_input_output_aliases=(),
                sim_require_finite=True,
                sim_require_nnan=True,
                nc=nc,
            )
        )

    devices = jax.devices()[:NCORES]
    mesh = Mesh(np.asarray(devices), ("core",))
    donate = tuple(range(n_params, n_params + n_outs))
    sharded = jax.jit(
        shard_map(
            _run_body,
            mesh=mesh,
            in_specs=(PartitionSpec("core"),) * (n_params + n_outs),
            out_specs=(PartitionSpec("core"),) * n_outs,
            check_rep=False,
        ),
        donate_argnums=donate,
        keep_unused=True,
    )

    def run(in_maps, timeit=False):
        concat_in = [
            np.concatenate([np.asarray(m[name]) for m in in_maps], axis=0)
            for name in in_names
        ]
        concat_in = [jax.device_put(a) for a in concat_in]
        zeros = [
            jax.device_put(
                np.zeros((NCORES * av.shape[0], *av.shape[1:]), av.dtype)
            )
            for av in out_avals
        ]
        for a in concat_in + zeros:
            a.block_until_ready()
        t0 = time.monotonic()
        out_arrs = sharded(*concat_in, *zeros)
        for o in out_arrs:
            o.block_until_ready()
        dt = time.monotonic() - t0
        res = [
            {
                name: np.asarray(out_arrs[i]).reshape(
                    NCORES, *out_avals[i].shape
                )[c]
                for i, name in enumerate(out_names)
            }
            for c in range(NCORES)
        ]
        if timeit:
            return res, dt
        return res

    _CACHE[key] = run
    return run


def make_in_maps(inputs):
    x = np.asarray(inputs["x"], np.float32).astype(ml_dtypes.bfloat16)
    shared = host_inputs(inputs)
    return [
        {"x": np.ascontiguousarray(x[c * BPC : (c + 1) * BPC]), **shared}
        for c in range(NCORES)
    ]


def kernel(**inputs):
    run = get_runner()
    in_maps = make_in_maps(inputs)
    res = run(in_maps)
    y = np.concatenate([np.asarray(r["y"]) for r in res], axis=0)
    return y.astype(np.float32)

